# revision 25
# baseline (speedup 1.0000x reference)
"""GRNN over perfect binary trees (jet embeddings) on 8 Trainium2 cores.

Strategy
--------
Host-side relabeling (as in the baseline) turns every gather into a
contiguous block read: order_0 = roots, order_{j+1} = [left children,
right children], so children of position p (of S) sit at p and S+p of
the next level.  The device kernel is then a pure matmul+activation
stream, leaf level -> root, fully SBUF-resident.

Device-side structure:
  * The K=7 u-projection is augmented to K=8 (ones row carries b_u) and
    row-tiled across 4 PE row-bands (partitions 0-7/32-39/64-71/96-103),
    four concurrent 256-col matmuls per 1024-col chunk -> ~4x fewer PE
    cycles on the u stream.
  * tanh is split across two engines.  ScalarE keeps the exact LUT tanh
    for all h-activations (b_h as free ACT bias) and for u at the 8
    root-adjacent levels.  A custom fused DVE op evaluates a clamped
    degree-5 odd polynomial approximation of tanh (Gaussian-weighted
    fit, input pre-scaled in the weights) for u at the 5 deepest levels;
    errors injected there decay geometrically level-by-level (measured
    end-to-end rel err ~1.5e-3).  This halves the ScalarE load, the
    baseline bottleneck.
  * Contents arrive as per-level band streams in ~64KB block DMAs (a
    handful of large transfers instead of hundreds of small ones).
  * A short burst of dummy matmuls at kernel start trips the PE HAM
    clock-gate so the real stream runs at 2.4 GHz from the beginning
    (the baseline spent half its runtime throttled at 1.2 GHz).
  * 1024-col chunks with 2+2 rotating PSUM tile pairs keep TensorE,
    ScalarE and VectorE concurrently busy.

Sharding: core d owns roots 8d..8d+8 -> 8 independent problems, no
collectives.
"""

import numpy as np
from contextlib import ExitStack

import concourse.bass as bass
import concourse.bacc as bacc
import concourse.tile as tile
from concourse import mybir
from concourse.bass_utils import run_bass_kernel_spmd

# ---- static problem geometry (hardcoded per contest rules) ----
B = 64
DEPTH = 12
N_FEAT = 7
AUGF = 8                 # features + ones row (bias)
N_HID = 128
N_CORES = 8
RPC = B // N_CORES       # roots per core

LEVEL_SIZES = [B * (1 << j) for j in range(DEPTH + 1)]
OFFSETS = np.concatenate([[0], np.cumsum(LEVEL_SIZES)]).astype(np.int64)
INNER_OFF = np.concatenate([[0], np.cumsum(LEVEL_SIZES[:-1])]).astype(np.int64)

PC_SIZES = {j: RPC << j for j in range(DEPTH + 1)}
PC_TOTAL = sum(PC_SIZES.values())  # 65528

CHUNK = 1024
NBANDS = 2               # PE row-bands used for the K=8 u-matmul
PIECE = CHUNK // NBANDS  # per-band piece of a chunk
BLK = 4096               # band-stream columns per staged DMA block
MMW = 512                # h-matmul free dim (one PSUM bank)
F16 = mybir.dt.float16
F32 = mybir.dt.float32

# clamped degree-5 odd polynomial ~ tanh:  p(y) = y*(1 + c1*t + c2*t^2),
# t = y^2, y = clip(a*x, -B, B); Gaussian-weighted L2 fit for x~N(0,1).
TA_A = 0.97451042
TA_B = 1.80329519
TA_C1 = -0.25736628
TA_C2 = 0.03575457
DVE_LEVELS = frozenset(range(8, DEPTH + 1))  # deep levels: poly-eligible

N_WARM_MM = 12           # ~4us of dummy matmuls to warm the HAM clock-gate


def _band_widths(S):
    """How a level of S columns is dealt across the PE row-bands."""
    if S >= CHUNK:
        return [S // NBANDS] * NBANDS
    widths = []
    rem = S
    while rem > 0 and len(widths) < NBANDS:
        w = min(PIECE, rem)
        widths.append(w)
        rem -= w
    return widths


# per-(level, band) offsets into the band-stream contents tensor
LEV_BAND_OFF = {}
_off = 0
for _j in range(DEPTH, -1, -1):
    for _q, _w in enumerate(_band_widths(PC_SIZES[_j])):
        LEV_BAND_OFF[(_j, _q)] = _off
        _off += _w
assert _off == PC_TOTAL

_COMPILED = {}


def _register_tanh_op():
    """Register the fused clamp+poly tanh approximation as a custom DVE op
    (the documented runtime extension point: append to dve_ops.OPS)."""
    import concourse.dve_ops as dvo
    from concourse.dve_spec import (
        Spec, Src0, C0, C1, C2, Zero, One, maxx, minn, sq, lower,
    )
    from concourse.dve_uop import DveOpSpec

    for op in dvo.OPS:
        if op.name == "TANH_POLY5_ANT":
            return op

    y = maxx(minn(Src0, C0), Zero - C0)
    t = sq(y)
    body = y * ((C2 * t + C1) * t + One)

    def ref(in0, in1, s0, s1, imm2):
        yy = np.clip(in0, -s0, s0).astype(np.float32)
        tt = yy * yy
        return (yy * ((imm2 * tt + s1) * tt + 1.0)).astype(np.float32)

    spec = Spec(body=body, reference=ref)
    row = dvo._CUSTOM_DVE_ROW_BASE + len(dvo.OPS)
    assert row < 0x20
    shas = {}
    for ver in ("v3", "v4"):
        s = DveOpSpec(name="TANH_POLY5_ANT", opcode=row,
                      uops=lower(spec, ver=ver), rd1_en=False)
        shas[ver] = s.sha(ver)
    op = dvo.DveOp("TANH_POLY5_ANT", spec, subdim=False, uops_sha=shas)
    dvo.OPS.append(op)
    dvo._SUB_OPCODE_FOR_NAME[op.name] = row
    dvo.CUSTOM_DVE_SPECS[op.name] = spec
    return op


def _emission_order():
    """A short act-bound prefix, then dense slots interleaving the
    remaining leaf chunks with level-11 work (the unbroken h-matmul
    stream keeps the PE clock-gate at 2.4 GHz while the leaf
    activations hide underneath), then the remaining levels serial."""
    order = [(DEPTH, ci) for ci in range(8)]
    order += [(DEPTH, 16 + ci) for ci in range(8)]
    for k in range(4):
        order += [(DEPTH, 8 + 2 * k), (DEPTH, 9 + 2 * k),
                  (DEPTH, 24 + 2 * k), (DEPTH, 25 + 2 * k)]
        order += [(DEPTH - 1, 4 * k + m) for m in range(4)]
    for j in range(DEPTH - 2, -1, -1):
        order += [(j, ci) for ci in range(max(1, PC_SIZES[j] // CHUNK))]
    return order


def _build_chunks():
    """Per-core chunk table in emission order.  Each chunk: level j, col
    offset a within the level, width w, band pieces (band q, width),
    u-activation engine and band-stream block index."""
    chunks = []
    for j, ci in _emission_order():
        S = PC_SIZES[j]
        bws = _band_widths(S)
        a = ci * CHUNK
        w = min(CHUNK, S - a)
        if w == CHUNK:
            pieces = [(q, PIECE) for q in range(NBANDS)]
        else:
            pieces = list(enumerate(bws))
        if j == DEPTH:
            eng = "dve" if ci % 2 == 0 else "se"
        elif j in DVE_LEVELS:
            eng = "dve"
        else:
            eng = "se"
        chunks.append(dict(j=j, ci=ci, a=a, w=w, pieces=pieces, eng=eng,
                           spos=ci, blk=(j, ci * PIECE // BLK)))
    return chunks


def _build_program():
    _register_tanh_op()
    nc = bacc.Bacc("TRN2", target_bir_lowering=False, debug=False,
                   num_devices=N_CORES)

    c_d = nc.dram_tensor("c", [AUGF, PC_TOTAL], F16, kind="ExternalInput").ap()
    wu_d = nc.dram_tensor("wu", [AUGF, 2 * N_HID], F16, kind="ExternalInput").ap()
    wh_d = nc.dram_tensor("wh", [N_HID, 3 * N_HID], F16, kind="ExternalInput").ap()
    bh_d = nc.dram_tensor("bh", [N_HID, 1], F32, kind="ExternalInput").ap()
    out_d = nc.dram_tensor("out", [N_HID, RPC], F32, kind="ExternalOutput").ap()

    with tile.TileContext(nc) as tc:
        with ExitStack() as ctx:
            _kernel_body(ctx, tc, c_d, wu_d, wh_d, bh_d, out_d)

    nc.compile()
    return nc


def _kernel_body(ctx, tc, c_d, wu_d, wh_d, bh_d, out_d):
    nc = tc.nc
    TANH = mybir.ActivationFunctionType.Tanh
    from concourse.dve_ops import OPS as _OPS
    tanh_op = next(op for op in _OPS if op.name == "TANH_POLY5_ANT")

    wpool = ctx.enter_context(tc.tile_pool(name="weights", bufs=1))
    epool = ctx.enter_context(tc.tile_pool(name="emb", bufs=1))
    cpool = ctx.enter_context(tc.tile_pool(name="cstage", bufs=3))
    upool = ctx.enter_context(tc.tile_pool(name="ustage", bufs=4))
    opool = ctx.enter_context(tc.tile_pool(name="outbuf", bufs=1))
    pupool = ctx.enter_context(tc.tile_pool(name="pu", bufs=2, space="PSUM"))
    phpool = ctx.enter_context(tc.tile_pool(name="ph", bufs=2, space="PSUM"))

    # boot the ScalarE activation-table load and the DVE uop tables at
    # t=0 so they hide under the initial DMA prefix
    boot_in = wpool.tile([N_HID, 16], F32)
    boot_out = wpool.tile([N_HID, 16], F16)
    nc.vector.memset(boot_in[:], 0.0)
    nc.scalar.activation(boot_out[:], boot_in[:],
                         mybir.ActivationFunctionType.Tanh)
    nc.vector._custom_dve(tanh_op, out=boot_out[:], in0=boot_in[:],
                          s0=TA_B, s1=TA_C1, imm2=TA_C2)

    # weights: u-projection pair (exact|prescaled) replicated per PE
    # row-band; h weights packed [whl|whr|whu] in one DMA.
    wu_sb = wpool.tile([32 * (NBANDS - 1) + AUGF, 2 * N_HID], F16)
    wh_sb = wpool.tile([N_HID, 3 * N_HID], F16)
    bh_sb = wpool.tile([N_HID, 1], F32)
    for q in range(NBANDS):
        nc.scalar.dma_start(wu_sb[32 * q:32 * q + AUGF, :], wu_d)
    nc.scalar.dma_start(wh_sb[:], wh_d)
    nc.scalar.dma_start(bh_sb[:], bh_d)
    whl_sb = wh_sb[:, 0 * N_HID:1 * N_HID]
    whr_sb = wh_sb[:, 1 * N_HID:2 * N_HID]
    whu_sb = wh_sb[:, 2 * N_HID:3 * N_HID]

    e_tiles = {}
    for j in range(DEPTH, 0, -1):
        e_tiles[j] = epool.tile([N_HID, PC_SIZES[j]], F16, name=f"e{j}")

    chunks = _build_chunks()

    # band-stream DMA blocks: trigger each block's DMAs ~8 chunks before
    # its first consumer so the transfer hides behind the pipeline
    first_chunk = {}
    for i, ch in enumerate(chunks):
        first_chunk.setdefault(ch["blk"], i)
    trigger = {}
    for blk, fc in first_chunk.items():
        trigger.setdefault(max(0, fc - 8), []).append(blk)
    blk_tiles = {}
    dma_engines = [nc.sync, nc.gpsimd]
    dma_rr = [0]

    def fetch_one(blk):
        if blk in blk_tiles:
            return
        j, g = blk
        if True:
            # rotation groups: interleaved emission means a shared rotating
            # tag across levels can FIFO-deadlock (a DMA waiting on consumers
            # that are queued behind a matmul waiting on that DMA).  The two
            # big levels get their own rotation; every small level gets a
            # dedicated buffer.
            if j == DEPTH:
                tag, bufs, width = "cstA", 3, BLK
            elif j == DEPTH - 1:
                tag, bufs, width = "cstB", 2, BLK
            else:
                tag, bufs, width = f"cst{j}", 1, max(_band_widths(PC_SIZES[j]))
            bt = cpool.tile([32 * (NBANDS - 1) + AUGF, width], F16, tag=tag,
                            bufs=bufs, name=f"cst_{j}_{g}")
            blk_tiles[blk] = bt
            for q, bw in enumerate(_band_widths(PC_SIZES[j])):
                lo = g * BLK
                wq = min(BLK, bw - lo)
                if wq <= 0:
                    continue
                src0 = LEV_BAND_OFF[(j, q)] + lo
                # first leaf block: halve the transfers so the very first
                # matmuls can start as early as possible
                parts = 2 if (j == DEPTH and g == 0) else 1
                step = (wq + parts - 1) // parts
                for p0 in range(0, wq, step):
                    pw = min(step, wq - p0)
                    eng_dma = dma_engines[dma_rr[0] % len(dma_engines)]
                    dma_rr[0] += 1
                    eng_dma.dma_start(bt[32 * q:32 * q + AUGF, p0:p0 + pw],
                                      c_d[:, src0 + p0:src0 + p0 + pw])

    def fetch_blocks(i):
        for blk in trigger.get(i, ()):
            fetch_one(blk)

    u_tiles = {}
    u_done = set()

    def emit_u(i):
        if i in u_done:
            return
        u_done.add(i)
        fetch_blocks(i)
        ch = chunks[i]
        j, a, w, eng = ch["j"], ch["a"], ch["w"], ch["eng"]
        fetch_one(ch["blk"])
        bt = blk_tiles[ch["blk"]]
        o = (ch["spos"] * PIECE) % BLK
        if j == DEPTH and ch["ci"] % 2 == 1:
            pu = phpool.tile([N_HID, CHUNK], F32, tag="ph", name=f"pu{i}")
        else:
            pu = pupool.tile([N_HID, CHUNK], F32, tag="pu", name=f"pu{i}")
        wlo = N_HID if eng == "dve" else 0
        for q, bw in ch["pieces"]:
            bp = 32 * q
            nc.tensor.matmul(pu[:, q * PIECE:q * PIECE + bw],
                             wu_sb[bp:bp + AUGF, wlo:wlo + N_HID],
                             bt[bp:bp + AUGF, o:o + bw],
                             start=True, stop=True, tile_position=(bp, 0))
        if j == DEPTH:
            dest = e_tiles[j][:, a:a + w]
        else:
            if j < 8:
                u_sb = upool.tile([N_HID, w], F16, tag=f"ushal{i}", bufs=1,
                                  name=f"u{i}")
            else:
                u_sb = upool.tile([N_HID, CHUNK], F16, tag="u", bufs=4,
                                  name=f"u{i}")
            u_tiles[i] = u_sb
            dest = u_sb[:, :w]
        if eng == "dve":
            nc.vector._custom_dve(tanh_op, out=dest, in0=pu[:, :w],
                                  s0=TA_B, s1=TA_C1, imm2=TA_C2)
        else:
            nc.scalar.activation(dest, pu[:, :w], TANH)

    shallow = [i for i, c in enumerate(chunks) if c["j"] < 8]
    c_start = next(i for i, c in enumerate(chunks) if c["j"] == DEPTH - 2)
    emit_u(0)
    emit_u(1)
    for i, ch in enumerate(chunks):
        if c_start <= i < c_start + len(shallow):
            # hoist shallow-level u work out of the serial root-ward tail
            # (u depends only on contents)
            emit_u(shallow[i - c_start])
        if i + 2 < len(chunks):
            emit_u(i + 2)
        j, a, w = ch["j"], ch["a"], ch["w"]
        if j == DEPTH:
            continue
        S = PC_SIZES[j]
        u_sb = u_tiles.pop(i)
        eprev = e_tiles[j + 1]
        ph = phpool.tile([N_HID, CHUNK], F32, tag="ph", name=f"ph{i}")
        # left children at level-(j+1) cols [a, a+w), right at [S+a, S+a+w);
        # grouped by stationary weight to keep the weight-load path cheap
        for s in range(0, w, MMW):
            bw = min(MMW, w - s)
            nc.tensor.matmul(ph[:, s:s + bw], whl_sb,
                             eprev[:, a + s:a + s + bw],
                             start=True, stop=False)
        for s in range(0, w, MMW):
            bw = min(MMW, w - s)
            nc.tensor.matmul(ph[:, s:s + bw], whr_sb,
                             eprev[:, S + a + s:S + a + s + bw],
                             start=False, stop=False)
        for s in range(0, w, MMW):
            bw = min(MMW, w - s)
            nc.tensor.matmul(ph[:, s:s + bw], whu_sb, u_sb[:, s:s + bw],
                             start=False, stop=True)
        if j > 0:
            nc.scalar.activation(e_tiles[j][:, a:a + w], ph[:, :w], TANH,
                                 bias=bh_sb[:, 0:1])
        else:
            out_sb = opool.tile([N_HID, RPC], F32)
            nc.scalar.activation(out_sb[:], ph[:, :RPC], TANH,
                                 bias=bh_sb[:, 0:1])
            nc.sync.dma_start(out_d, out_sb[:])


def _preprocess(contents, children):
    """Relabel nodes so children of position p live at p, S+p; return
    per-core fp16 contents in per-level 4-band-stream order with a
    trailing ones row for the bias."""
    contents = np.asarray(contents, dtype=np.float32)
    children = np.asarray(children)
    clipped = []
    for j in range(DEPTH):
        ch = children[INNER_OFF[j]:INNER_OFF[j + 1]]
        clipped.append(np.clip(ch, 0, LEVEL_SIZES[j + 1] - 1).astype(np.int64))

    per_core = []
    for d in range(N_CORES):
        o = np.arange(d * RPC, (d + 1) * RPC, dtype=np.int64)
        segs = [contents[OFFSETS[0] + o]]
        for j in range(DEPTH):
            sel = clipped[j][o]
            o = np.concatenate([sel[:, 0], sel[:, 1]])
            segs.append(contents[OFFSETS[j + 1] + o])
        segs.reverse()                      # leaf level first
        Ca = np.empty((AUGF, PC_TOTAL), np.float16)
        pos = 0
        for li, j in enumerate(range(DEPTH, -1, -1)):
            L = segs[li].T.astype(np.float16)   # [7, S]
            S = L.shape[1]
            if S >= CHUNK:
                nch = S // CHUNK
                L = (L.reshape(N_FEAT, nch, NBANDS, PIECE)
                      .transpose(0, 2, 1, 3).reshape(N_FEAT, S))
            Ca[:N_FEAT, pos:pos + S] = L
            pos += S
        Ca[N_FEAT] = np.float16(1.0)
        per_core.append(np.ascontiguousarray(Ca))
    return per_core


def kernel(contents, children, w_u, b_u, w_h, b_h):
    contents = np.asarray(contents)
    children = np.asarray(children)
    w_u = np.asarray(w_u, dtype=np.float32)
    b_u = np.asarray(b_u, dtype=np.float32)
    w_h = np.asarray(w_h, dtype=np.float32)
    b_h = np.asarray(b_h, dtype=np.float32)

    per_core_c = _preprocess(contents, children)

    wue = np.empty((AUGF, N_HID), np.float32)
    wue[:N_FEAT] = w_u.T
    wue[N_FEAT] = b_u
    wu_t = np.empty((AUGF, 2 * N_HID), np.float16)
    wu_t[:, :N_HID] = wue.astype(np.float16)
    wu_t[:, N_HID:] = (wue * np.float32(TA_A)).astype(np.float16)
    wh_t = np.empty((N_HID, 3 * N_HID), np.float16)
    wh_t[:, 0:128] = w_h[:, 0:128].T.astype(np.float16)
    wh_t[:, 128:256] = w_h[:, 128:256].T.astype(np.float16)
    wh_t[:, 256:384] = w_h[:, 256:384].T.astype(np.float16)
    bh_c = np.ascontiguousarray(b_h.reshape(N_HID, 1))

    if "nc" not in _COMPILED:
        _COMPILED["nc"] = _build_program()
    nc = _COMPILED["nc"]

    in_maps = []
    for d in range(N_CORES):
        in_maps.append({
            "c": per_core_c[d],
            "wu": wu_t, "wh": wh_t, "bh": bh_c,
        })
    res = run_bass_kernel_spmd(nc, in_maps, list(range(N_CORES)))

    out = np.empty((B, N_HID), dtype=np.float32)
    for d in range(N_CORES):
        out[d * RPC:(d + 1) * RPC, :] = res.results[d]["out"].T
    return out
